# revision 18
# baseline (speedup 1.0000x reference)
"""Causal single-head attention (Q==K source bug faithful) on 8 TRN2 NeuronCores.

Problem: x [4, 4096, 1024], Wk/Wv [1024, 64];
  k = q = x@Wk; scores = q k^T / 8, causal softmax, out = weights @ (x@Wv).

Strategy (no collectives, uniform SPMD program):
  - 8 cores = 4 batches x 2 roles. Each core computes the full K/V
    projection for its batch (redundantly) and owns 2 query chunks of
    1024 rows: role A owns global chunks {0, 3}, role B owns {1, 2}.
    This balances causal attention work exactly (pairs: 1+7 == 3+5).
  - One compiled program for all cores; per-core differences are pure
    data (host permutes key panels; a 0/1 vmask zeroes V columns of
    keys a chunk must not see, so padding slots contribute zero to
    numerator and denominator; softmax without max-subtraction is safe:
    scores <= ~16).
  - Flash-style transposed layout: scores^T [keys_p, queries_f] via
    TensorE, exp on ScalarE (scale=1/8 fused), P@V via TensorE with
    ones-row V giving the denominator free, PSUM accumulation across
    key blocks, PE-transpose epilogue + reciprocal multiply.
  - v2 perf restructure: chunks processed sequentially (one PSUM acc
    live at a time) freeing banks for a dedicated projection pool;
    host supplies bf16 (halves DMA, no on-chip casts); first panels'
    DMA is chi-sliced so the PE starts ~1.5us in; projection matmuls
    are interleaved into the attention slot stream as PE filler so the
    PE never idles (keeps the DVFS p-state at max 2.4GHz instead of
    1.2GHz); PSUM->SBUF copies moved to GpSimd; diagonal causal masks
    shrunk to one 128-col triangle multiply on VectorE (4x bf16 mode).
"""
import numpy as np
import ml_dtypes

import concourse.bass as bass
import concourse.mybir as mybir
from concourse import bacc, tile
from concourse.bass_utils import run_bass_kernel_spmd

F32 = mybir.dt.float32
BF16 = mybir.dt.bfloat16
EXP = mybir.ActivationFunctionType.Exp

B, T, C, H = 4, 4096, 1024, 64
NCHI = C // 128          # 8 contraction blocks
NPAN = 8                 # 512-row key panels per core
PAN = 512
NKB = 32                 # 128-row key blocks per core
CHUNK = 1024             # queries per chunk
ROLE_CHUNKS = {0: (0, 3), 1: (1, 2)}


def pieces(c0):
    """Split [c0, 1024) at the PSUM bank boundary (512 f32)."""
    if c0 >= 512:
        return [(c0, 1024 - c0)]
    return [(c0, 512 - c0), (512, 512)]


def build_nc():
    nc = bacc.Bacc("TRN2", target_bir_lowering=False, debug=False, num_devices=8)

    xt_d = nc.declare_dram_parameter("xt", [128, NPAN, NCHI, PAN], BF16, isOutput=False)
    wkv_d = nc.declare_dram_parameter("wkv", [128, NCHI, 128], BF16, isOutput=False)
    vm_d = nc.declare_dram_parameter("vm", [128, 2 * NKB], F32, isOutput=False)
    mk_d = nc.declare_dram_parameter("mk", [128, 128], BF16, isOutput=False)
    eyb_d = nc.declare_dram_parameter("eyb", [64, 64], F32, isOutput=False)
    eyf_d = nc.declare_dram_parameter("eyf", [65, 65], F32, isOutput=False)
    out_d = nc.declare_dram_parameter("out", [2 * CHUNK, H], F32, isOutput=True)
    out_v = out_d.ap().rearrange("(i p) h -> p i h", p=128)  # [128, 16, 64]

    with tile.TileContext(nc) as tc:
        with (
            tc.tile_pool(name="const", bufs=1) as const,
            tc.tile_pool(name="xt", bufs=12) as xtp,
            tc.tile_pool(name="vh", bufs=2) as vhp,
            tc.tile_pool(name="pt", bufs=6) as ptp,
            tc.tile_pool(name="osb", bufs=2) as osbp,
            tc.tile_pool(name="outsb", bufs=2) as otp,
            tc.tile_pool(name="rc", bufs=4) as rcp,
            tc.tile_pool(name="psS", bufs=2, space="PSUM") as psS,
            tc.tile_pool(name="psK", bufs=2, space="PSUM") as psK,
            tc.tile_pool(name="psO", bufs=1, space="PSUM") as psO,
        ):
            wkv = const.tile([128, NCHI, 128], BF16, tag="wkv")
            vm = const.tile([128, 2 * NKB], F32, tag="vm")
            mk = const.tile([128, 128], BF16, tag="mk")
            eyb = const.tile([64, 64], F32, tag="eyb")
            eyf = const.tile([65, 65], F32, tag="eyf")
            kt = const.tile([64, T], BF16, tag="kt")            # K^T, permuted cols
            vaug = const.tile([128, NKB, 65], BF16, tag="vaug")   # chunk0-masked V|1
            vaug2 = const.tile([128, NKB, 65], BF16, tag="vaug2")  # chunk1-masked V|1
            vaugs = (vaug, vaug2)

            nc.sync.dma_start(wkv[:, 0:2], wkv_d[:, 0:2])
            nc.sync.dma_start(wkv[:, 2:NCHI], wkv_d[:, 2:NCHI])

            xts = {}

            def dma_panel(p):
                """Four 2-chi subtiles per panel, alternating DMA queues, so
                the projection waits only on the slice it consumes."""
                subs = []
                for q in range(4):
                    xt = xtp.tile([128, 2, PAN], BF16, tag="xt", name=f"xt{p}_{q}")
                    eng = nc.sync if q % 2 == 0 else nc.gpsimd
                    eng.dma_start(xt[:], xt_d[:, p, 2 * q:2 * q + 2])
                    subs.append(xt)
                xts[p] = subs

            kvs = {}

            def proj(p, ci):
                """One contraction step of panel p's K/V projection."""
                if ci == 0:
                    kvs[p] = psK.tile([128, PAN], F32, tag="kv", name=f"kv{p}")
                nc.tensor.matmul(
                    kvs[p][:], wkv[:, ci, :], xts[p][ci // 2][:, ci % 2, :],
                    start=(ci == 0), stop=(ci == NCHI - 1),
                )
                if ci == NCHI - 1:
                    kv = kvs[p]
                    nc.vector.tensor_copy(kt[:, p * PAN:(p + 1) * PAN], kv[0:64, :])
                    vh = vhp.tile([64, PAN], F32, tag="vh")
                    nc.vector.tensor_copy(vh[:], kv[64:128, :])
                    kvs[p] = vh

            def vtrans(p):
                """Transpose V panel and build both masked V|1 variants."""
                vh = kvs.pop(p)
                kv2 = psK.tile([128, PAN], F32, tag="kv")
                for tb in range(4):
                    nc.tensor.transpose(
                        kv2[:, tb * 64:(tb + 1) * 64],
                        vh[:, tb * 128:(tb + 1) * 128], eyb,
                    )
                vv = kv2[:, 0:256].rearrange("p (a b) -> p a b", a=4)
                for which, vt in enumerate(vaugs):
                    vcol = vm[:, which * NKB + 4 * p: which * NKB + 4 * p + 4]
                    nc.vector.tensor_mul(
                        vt[:, 4 * p:4 * p + 4, 0:64],
                        vv,
                        vcol.unsqueeze(2).broadcast_to([128, 4, 64]),
                    )
                    nc.vector.tensor_copy(
                        vt[:, 4 * p:4 * p + 4, 64:65], vcol.unsqueeze(2)
                    )

            acc = [None]

            def slot(chunk, kb, c0=0, first=False, stop0=False, stop1=False,
                     diag=False):
                """One key-block slot: scores^T -> exp -> (mask) -> P@V."""
                Q = kt[:, chunk * CHUNK:(chunk + 1) * CHUNK]
                ps = pieces(c0)
                s_ps = psS.tile([128, 1024], F32, tag="ps")
                for (o, ln) in ps:
                    nc.tensor.matmul(
                        s_ps[:, o:o + ln],
                        kt[:, kb * 128:(kb + 1) * 128],
                        Q[:, o:o + ln],
                        start=True, stop=True,
                    )
                pt = ptp.tile([128, 1024], BF16, tag="pt")
                nc.scalar.activation(pt[:, c0:1024], s_ps[:, c0:1024], EXP, scale=0.125)
                if diag:
                    nc.vector.tensor_mul(pt[:, c0:c0 + 128], pt[:, c0:c0 + 128], mk[:])
                if first:
                    acc[0] = psO.tile([65, 1024], F32, tag="ot", name=f"ot{chunk}")
                for (o, ln) in ps:
                    nc.tensor.matmul(
                        acc[0][:, o:o + ln],
                        vaugs[chunk % 2][:, kb, :],
                        pt[:, o:o + ln],
                        start=first, stop=(stop0 if o < 512 else stop1),
                    )

            def ep_copy(chunk):
                """Copy the PSUM accumulator out so the acc pool can rotate."""
                a = acc[0]
                osb = osbp.tile([65, 1024], F32, tag="osb")
                for half in range(2):
                    nc.vector.tensor_copy(
                        osb[:, half * 512:(half + 1) * 512],
                        a[:, half * 512:(half + 1) * 512],
                    )
                return osb

            def epilogue(chunk, osb):
                """Per-bank: transpose 128-query blocks, divide by the
                transposed denominator column, DMA out.  Chunk 1's transposes
                use the scores pool (idle by then) so they never wait on pool
                rotation in the tail."""
                ci = chunk % 2
                outsb = otp.tile([128, 8, H], F32, tag="outsb")
                for half in range(2):
                    for k in range(2):
                        if ci == 1:
                            tile_ = psS.tile([128, 1024], F32, tag="ps")
                        else:
                            tile_ = psK.tile([128, PAN], F32, tag="kv")
                        for j in range(2):
                            i = 4 * half + 2 * k + j
                            nc.tensor.transpose(
                                tile_[:, 256 * j:256 * j + 65],
                                osb[:, i * 128:(i + 1) * 128], eyf,
                            )
                        for j in range(2):
                            i = 4 * half + 2 * k + j
                            rc = rcp.tile([128, 1], F32, tag="rc")
                            nc.vector.reciprocal(
                                rc[:], tile_[:, 256 * j + 64:256 * j + 65]
                            )
                            nc.vector.tensor_scalar_mul(
                                outsb[:, i, :], tile_[:, 256 * j:256 * j + 64],
                                rc[:],
                            )
                    nc.sync.dma_start(
                        out_v[:, 8 * ci + 4 * half:8 * ci + 4 * half + 4, :],
                        outsb[:, 4 * half:4 * half + 4, :],
                    )

            # ---- schedule ----
            # head: panels 0,1,4 fully projected before the slot stream.
            # Consts ride the gpsimd queue after the first panels so they
            # never delay the projection-critical slices.
            dma_panel(0)
            dma_panel(1)
            nc.gpsimd.dma_start(eyb[:], eyb_d[:])
            nc.gpsimd.dma_start(vm[:], vm_d[:])
            dma_panel(4)
            nc.gpsimd.dma_start(mk[:], mk_d[:])
            nc.gpsimd.dma_start(eyf[:], eyf_d[:])
            for ci in range(NCHI):
                proj(0, ci)
            for ci in range(NCHI):
                proj(1, ci)
            vtrans(0)
            vtrans(1)
            for ci in range(NCHI):
                proj(4, ci)
            vtrans(4)

            # filler queue: (panel, unit) where unit 0..7 = proj ci, 8 = vtrans
            fill = []
            for p in (5, 2, 3, 6, 7):
                fill += [(p, u) for u in range(9)]
            fidx = [0]

            def filler(n):
                for _ in range(n):
                    if fidx[0] >= len(fill):
                        return
                    p, u = fill[fidx[0]]
                    fidx[0] += 1
                    if u == 8:
                        vtrans(p)
                    else:
                        proj(p, u)

            # chunk 0: diag m0..3, off-diag kb16..23, diag m4..7 last.
            # bank0 closes at kb23 (slot 11), bank1 at m7 (slot 15) so the
            # epilogue halves overlap the tail slots.
            slots0 = [dict(kb=m, c0=128 * m, diag=True) for m in range(4)]
            slots0 += [dict(kb=kb) for kb in range(16, 24)]
            slots0 += [dict(kb=m, c0=128 * m, diag=True) for m in range(4, 8)]
            slots0[0]["first"] = True
            slots0[11]["stop0"] = True
            slots0[15]["stop1"] = True
            # chunk 1: kb0..7, kb16..23, kb24..31, diag m0..7 last.
            slots1 = [dict(kb=kb) for kb in range(0, 8)]
            slots1 += [dict(kb=kb) for kb in range(16, 24)]
            slots1 += [dict(kb=kb) for kb in range(24, 32)]
            slots1 += [dict(kb=8 + m, c0=128 * m, diag=True) for m in range(8)]
            slots1[0]["first"] = True
            slots1[27]["stop0"] = True   # diag m3: last bank0 writer
            slots1[31]["stop1"] = True   # diag m7: last bank1 writer
            for sl in slots1[28:31]:
                assert sl["c0"] >= 512    # m4..6 never touch bank0

            # pacing: p5 by slot 8, p2/p3 by 16, p6 by 32, p7 by 36
            pace = {**{i: 2 for i in range(12)}, **{i: 1 for i in range(12, 34)}}
            dma_at = {0: 5, 2: 2, 6: 3, 13: 6, 22: 7}

            for i, s in enumerate(slots0):
                slot(0, **s)
                if i in dma_at:
                    dma_panel(dma_at[i])
                filler(pace.get(i, 0))
            osb0 = ep_copy(0)
            for j, s in enumerate(slots1):
                i = j + 16
                slot(1, **s)
                if i in dma_at:
                    dma_panel(dma_at[i])
                filler(pace.get(i, 0))
                if i == 17:
                    epilogue(0, osb0)   # PE filler for the Scalar-bound tail
            filler(len(fill))
            osb1 = ep_copy(1)
            epilogue(1, osb1)

    nc.compile()
    return nc


def make_inputs(x, Wk, Wv):
    """Build the 8 per-core input maps (pure layout work, host side)."""
    bf16 = ml_dtypes.bfloat16
    wkv = np.concatenate([Wk, Wv], axis=1)            # [1024, 128]
    wkv_t = wkv.reshape(NCHI, 128, 128).transpose(1, 0, 2).astype(bf16)

    pp = np.arange(128)[:, None]
    jj = np.arange(128)[None, :]
    mk = (jj >= pp).astype(bf16)                      # [128,128] triangle

    eyb = np.eye(64, dtype=np.float32)
    eyf = np.eye(65, dtype=np.float32)

    in_maps = []
    for c in range(8):
        b, role = divmod(c, 2)
        lo_g, hi_g = ROLE_CHUNKS[role]
        others = sorted(set(range(4)) - {lo_g, hi_g})
        pan = [2 * lo_g, 2 * lo_g + 1, 2 * hi_g, 2 * hi_g + 1]
        for o in others:
            pan += [2 * o, 2 * o + 1]

        xT = np.ascontiguousarray(x[b].T)             # [1024, 4096]
        xr = xT.reshape(NCHI, 128, T)                 # [chi, cp, t]
        xt = np.empty((128, NPAN, NCHI, PAN), dtype=bf16)
        for j, pg in enumerate(pan):
            xt[:, j] = xr[:, :, pg * PAN:(pg + 1) * PAN].transpose(1, 0, 2)

        gstart = np.empty(NKB, dtype=np.int64)        # global row of each kb
        for kb in range(NKB):
            gstart[kb] = pan[kb // 4] * PAN + (kb % 4) * 128
        vmask = np.zeros((128, 2 * NKB), dtype=np.float32)
        vmask[:, 0:NKB] = (gstart < (lo_g + 1) * CHUNK).astype(np.float32)[None, :]
        vmask[:, NKB:] = (gstart < (hi_g + 1) * CHUNK).astype(np.float32)[None, :]

        in_maps.append(
            {"xt": xt, "wkv": wkv_t, "vm": vmask, "mk": mk, "eyb": eyb,
             "eyf": eyf}
        )
    return in_maps


_NC = None


def get_nc():
    global _NC
    if _NC is None:
        _NC = build_nc()
    return _NC


def kernel(x, Wk, Wv):
    x = np.asarray(x, dtype=np.float32)
    Wk = np.asarray(Wk, dtype=np.float32)
    Wv = np.asarray(Wv, dtype=np.float32)
    nc = get_nc()
    in_maps = make_inputs(x, Wk, Wv)
    res = run_bass_kernel_spmd(nc, in_maps, list(range(8)))
    out = np.empty((B, T, H), dtype=np.float32)
    for c in range(8):
        b, role = divmod(c, 2)
        lo_g, hi_g = ROLE_CHUNKS[role]
        o = res.results[c]["out"]
        out[b, lo_g * CHUNK:(lo_g + 1) * CHUNK] = o[0:CHUNK]
        out[b, hi_g * CHUNK:(hi_g + 1) * CHUNK] = o[CHUNK:]
    return out


# revision 19
# speedup vs baseline: 1.0880x; 1.0880x over previous
"""Causal single-head attention (Q==K source bug faithful) on 8 TRN2 NeuronCores.

Problem: x [4, 4096, 1024], Wk/Wv [1024, 64];
  k = q = x@Wk; scores = q k^T / 8, causal softmax, out = weights @ (x@Wv).

Strategy (no collectives, uniform SPMD program):
  - 8 cores = 4 batches x 2 roles. Each core computes the full K/V
    projection for its batch (redundantly) and owns 2 query chunks of
    1024 rows: role A owns global chunks {0, 3}, role B owns {1, 2}.
    This balances causal attention work exactly (pairs: 1+7 == 3+5).
  - One compiled program for all cores; per-core differences are pure
    data (host permutes key panels; a 0/1 vmask zeroes V columns of
    keys a chunk must not see, so padding slots contribute zero to
    numerator and denominator; softmax without max-subtraction is safe:
    scores <= ~16).
  - Flash-style transposed layout: scores^T [keys_p, queries_f] via
    TensorE, exp on ScalarE (scale=1/8 fused), P@V via TensorE with
    ones-row V giving the denominator free, PSUM accumulation across
    key blocks, PE-transpose epilogue + reciprocal multiply.
  - v2 perf restructure: chunks processed sequentially (one PSUM acc
    live at a time) freeing banks for a dedicated projection pool;
    host supplies bf16 (halves DMA, no on-chip casts); first panels'
    DMA is chi-sliced so the PE starts ~1.5us in; projection matmuls
    are interleaved into the attention slot stream as PE filler so the
    PE never idles (keeps the DVFS p-state at max 2.4GHz instead of
    1.2GHz); PSUM->SBUF copies moved to GpSimd; diagonal causal masks
    shrunk to one 128-col triangle multiply on VectorE (4x bf16 mode).
"""
import numpy as np
import ml_dtypes

import concourse.bass as bass
import concourse.mybir as mybir
from concourse import bacc, tile
from concourse.bass_utils import run_bass_kernel_spmd

F32 = mybir.dt.float32
BF16 = mybir.dt.bfloat16
EXP = mybir.ActivationFunctionType.Exp

B, T, C, H = 4, 4096, 1024, 64
NCHI = C // 128          # 8 contraction blocks
NPAN = 8                 # 512-row key panels per core
PAN = 512
NKB = 32                 # 128-row key blocks per core
CHUNK = 1024             # queries per chunk
ROLE_CHUNKS = {0: (0, 3), 1: (1, 2)}


def pieces(c0):
    """Split [c0, 1024) at the PSUM bank boundary (512 f32)."""
    if c0 >= 512:
        return [(c0, 1024 - c0)]
    return [(c0, 512 - c0), (512, 512)]


def build_nc():
    nc = bacc.Bacc("TRN2", target_bir_lowering=False, debug=False, num_devices=8)

    xt_d = nc.declare_dram_parameter("xt", [128, NPAN, NCHI, PAN], BF16, isOutput=False)
    wkv_d = nc.declare_dram_parameter("wkv", [128, NCHI, 128], BF16, isOutput=False)
    vm_d = nc.declare_dram_parameter("vm", [128, 2 * NKB], F32, isOutput=False)
    mk_d = nc.declare_dram_parameter("mk", [128, 128], BF16, isOutput=False)
    eyb_d = nc.declare_dram_parameter("eyb", [64, 64], F32, isOutput=False)
    eyf_d = nc.declare_dram_parameter("eyf", [65, 65], F32, isOutput=False)
    out_d = nc.declare_dram_parameter("out", [2 * CHUNK, H], F32, isOutput=True)
    out_v = out_d.ap().rearrange("(i p) h -> p i h", p=128)  # [128, 16, 64]

    with tile.TileContext(nc) as tc:
        with (
            tc.tile_pool(name="const", bufs=1) as const,
            tc.tile_pool(name="xt", bufs=12) as xtp,
            tc.tile_pool(name="vh", bufs=2) as vhp,
            tc.tile_pool(name="pt", bufs=6) as ptp,
            tc.tile_pool(name="osb", bufs=2) as osbp,
            tc.tile_pool(name="outsb", bufs=2) as otp,
            tc.tile_pool(name="rc", bufs=4) as rcp,
            tc.tile_pool(name="psS", bufs=2, space="PSUM") as psS,
            tc.tile_pool(name="psK", bufs=2, space="PSUM") as psK,
            tc.tile_pool(name="psO", bufs=1, space="PSUM") as psO,
        ):
            wkv = const.tile([128, NCHI, 128], BF16, tag="wkv")
            vm = const.tile([128, 2 * NKB], F32, tag="vm")
            mk = const.tile([128, 128], BF16, tag="mk")
            eyb = const.tile([64, 64], F32, tag="eyb")
            eyf = const.tile([65, 65], F32, tag="eyf")
            kt = const.tile([64, T], BF16, tag="kt")            # K^T, permuted cols
            vaug = const.tile([128, NKB, 65], BF16, tag="vaug")   # chunk0-masked V|1
            vaug2 = const.tile([128, NKB, 65], BF16, tag="vaug2")  # chunk1-masked V|1
            vaugs = (vaug, vaug2)

            nc.sync.dma_start(wkv[:, 0:2], wkv_d[:, 0:2])
            nc.sync.dma_start(wkv[:, 2:NCHI], wkv_d[:, 2:NCHI])

            xts = {}

            def dma_panel(p, fine=False):
                """Subtile DMAs alternating across both queues so the
                projection waits only on the slice it consumes.  fine=True
                uses 1-chi slices (panel 0: fastest first arrival)."""
                subs = []
                per = 1 if fine else 2
                for q in range(NCHI // per):
                    xt = xtp.tile([128, per, PAN], BF16, tag=f"xt{per}",
                                  name=f"xt{p}_{q}")
                    eng = nc.sync if q % 2 == 0 else nc.gpsimd
                    eng.dma_start(xt[:], xt_d[:, p, per * q:per * q + per])
                    subs.append(xt)
                xts[p] = (subs, per)

            kvs = {}

            def proj(p, ci):
                """One contraction step of panel p's K/V projection."""
                if ci == 0:
                    kvs[p] = psK.tile([128, PAN], F32, tag="kv", name=f"kv{p}")
                subs, per = xts[p]
                nc.tensor.matmul(
                    kvs[p][:], wkv[:, ci, :], subs[ci // per][:, ci % per, :],
                    start=(ci == 0), stop=(ci == NCHI - 1),
                )
                if ci == NCHI - 1:
                    kv = kvs[p]
                    nc.vector.tensor_copy(kt[:, p * PAN:(p + 1) * PAN], kv[0:64, :])
                    vh = vhp.tile([64, PAN], F32, tag="vh")
                    nc.vector.tensor_copy(vh[:], kv[64:128, :])
                    kvs[p] = vh

            def vtrans(p):
                """Transpose V panel and build both masked V|1 variants."""
                vh = kvs.pop(p)
                kv2 = psK.tile([128, PAN], F32, tag="kv")
                for tb in range(4):
                    nc.tensor.transpose(
                        kv2[:, tb * 64:(tb + 1) * 64],
                        vh[:, tb * 128:(tb + 1) * 128], eyb,
                    )
                vv = kv2[:, 0:256].rearrange("p (a b) -> p a b", a=4)
                for which, vt in enumerate(vaugs):
                    vcol = vm[:, which * NKB + 4 * p: which * NKB + 4 * p + 4]
                    nc.vector.tensor_mul(
                        vt[:, 4 * p:4 * p + 4, 0:64],
                        vv,
                        vcol.unsqueeze(2).broadcast_to([128, 4, 64]),
                    )
                    nc.vector.tensor_copy(
                        vt[:, 4 * p:4 * p + 4, 64:65], vcol.unsqueeze(2)
                    )

            acc = [None]

            def slot(chunk, kb, c0=0, first=False, stop0=False, stop1=False,
                     diag=False):
                """One key-block slot: scores^T -> exp -> (mask) -> P@V."""
                Q = kt[:, chunk * CHUNK:(chunk + 1) * CHUNK]
                ps = pieces(c0)
                s_ps = psS.tile([128, 1024], F32, tag="ps")
                for (o, ln) in ps:
                    nc.tensor.matmul(
                        s_ps[:, o:o + ln],
                        kt[:, kb * 128:(kb + 1) * 128],
                        Q[:, o:o + ln],
                        start=True, stop=True,
                    )
                pt = ptp.tile([128, 1024], BF16, tag="pt")
                nc.scalar.activation(pt[:, c0:1024], s_ps[:, c0:1024], EXP, scale=0.125)
                if diag:
                    nc.vector.tensor_mul(pt[:, c0:c0 + 128], pt[:, c0:c0 + 128], mk[:])
                if first:
                    acc[0] = psO.tile([65, 1024], F32, tag="ot", name=f"ot{chunk}")
                for (o, ln) in ps:
                    nc.tensor.matmul(
                        acc[0][:, o:o + ln],
                        vaugs[chunk % 2][:, kb, :],
                        pt[:, o:o + ln],
                        start=first, stop=(stop0 if o < 512 else stop1),
                    )

            def ep_copy(chunk):
                """Copy the PSUM accumulator out so the acc pool can rotate."""
                a = acc[0]
                osb = osbp.tile([65, 1024], F32, tag="osb")
                for half in range(2):
                    nc.vector.tensor_copy(
                        osb[:, half * 512:(half + 1) * 512],
                        a[:, half * 512:(half + 1) * 512],
                    )
                return osb

            def epilogue(chunk, osb):
                """Per-bank: transpose 128-query blocks, divide by the
                transposed denominator column, DMA out.  Chunk 1's transposes
                use the scores pool (idle by then) so they never wait on pool
                rotation in the tail."""
                ci = chunk % 2
                outsb = otp.tile([128, 8, H], F32, tag="outsb")
                for half in range(2):
                    for k in range(2):
                        if ci == 1:
                            tile_ = psS.tile([128, 1024], F32, tag="ps")
                        else:
                            tile_ = psK.tile([128, PAN], F32, tag="kv")
                        for j in range(2):
                            i = 4 * half + 2 * k + j
                            nc.tensor.transpose(
                                tile_[:, 256 * j:256 * j + 65],
                                osb[:, i * 128:(i + 1) * 128], eyf,
                            )
                        for j in range(2):
                            i = 4 * half + 2 * k + j
                            rc = rcp.tile([128, 1], F32, tag="rc")
                            nc.vector.reciprocal(
                                rc[:], tile_[:, 256 * j + 64:256 * j + 65]
                            )
                            nc.vector.tensor_scalar_mul(
                                outsb[:, i, :], tile_[:, 256 * j:256 * j + 64],
                                rc[:],
                            )
                    nc.sync.dma_start(
                        out_v[:, 8 * ci + 4 * half:8 * ci + 4 * half + 4, :],
                        outsb[:, 4 * half:4 * half + 4, :],
                    )

            # ---- schedule ----
            # head: panels 0,1,4 fully projected before the slot stream.
            # Consts ride the gpsimd queue after the first panels so they
            # never delay the projection-critical slices.
            dma_panel(0, fine=True)
            dma_panel(1)
            nc.gpsimd.dma_start(eyb[:], eyb_d[:])
            nc.gpsimd.dma_start(vm[:], vm_d[:])
            dma_panel(4)
            nc.gpsimd.dma_start(mk[:], mk_d[:])
            nc.gpsimd.dma_start(eyf[:], eyf_d[:])
            for ci in range(NCHI):
                proj(0, ci)
            for ci in range(NCHI):
                proj(1, ci)
            vtrans(0)
            vtrans(1)
            for ci in range(NCHI):
                proj(4, ci)
            vtrans(4)

            # filler queue: (panel, unit) where unit 0..7 = proj ci, 8 = vtrans
            fill = []
            for p in (5, 2, 3, 6, 7):
                fill += [(p, u) for u in range(9)]
            fidx = [0]

            def filler(n):
                for _ in range(n):
                    if fidx[0] >= len(fill):
                        return
                    p, u = fill[fidx[0]]
                    fidx[0] += 1
                    if u == 8:
                        vtrans(p)
                    else:
                        proj(p, u)

            # chunk 0: diag m0..3, off-diag kb16..23, diag m4..7 last.
            # bank0 closes at kb23 (slot 11), bank1 at m7 (slot 15) so the
            # epilogue halves overlap the tail slots.
            slots0 = [dict(kb=m, c0=128 * m, diag=True) for m in range(4)]
            slots0 += [dict(kb=kb) for kb in range(16, 24)]
            slots0 += [dict(kb=m, c0=128 * m, diag=True) for m in range(4, 8)]
            slots0[0]["first"] = True
            slots0[11]["stop0"] = True
            slots0[15]["stop1"] = True
            # chunk 1: kb0..7, kb16..23, kb24..31, diag m0..7 last.
            slots1 = [dict(kb=kb) for kb in range(0, 8)]
            slots1 += [dict(kb=kb) for kb in range(16, 24)]
            slots1 += [dict(kb=kb) for kb in range(24, 32)]
            slots1 += [dict(kb=8 + m, c0=128 * m, diag=True) for m in range(8)]
            slots1[0]["first"] = True
            slots1[27]["stop0"] = True   # diag m3: last bank0 writer
            slots1[31]["stop1"] = True   # diag m7: last bank1 writer
            for sl in slots1[28:31]:
                assert sl["c0"] >= 512    # m4..6 never touch bank0

            # pacing: p5 by slot 8, p2/p3 by 16, p6 by 32, p7 by 36
            pace = {**{i: 2 for i in range(12)}, **{i: 1 for i in range(12, 34)}}
            dma_at = {0: 5, 2: 2, 6: 3, 13: 6, 22: 7}

            for i, s in enumerate(slots0):
                slot(0, **s)
                if i in dma_at:
                    dma_panel(dma_at[i])
                filler(pace.get(i, 0))
            osb0 = ep_copy(0)
            for j, s in enumerate(slots1):
                i = j + 16
                slot(1, **s)
                if i in dma_at:
                    dma_panel(dma_at[i])
                filler(pace.get(i, 0))
                if i == 17:
                    epilogue(0, osb0)   # PE filler for the Scalar-bound tail
            filler(len(fill))
            osb1 = ep_copy(1)
            epilogue(1, osb1)

    nc.compile()
    return nc


def make_inputs(x, Wk, Wv):
    """Build the 8 per-core input maps (pure layout work, host side)."""
    bf16 = ml_dtypes.bfloat16
    wkv = np.concatenate([Wk, Wv], axis=1)            # [1024, 128]
    wkv_t = wkv.reshape(NCHI, 128, 128).transpose(1, 0, 2).astype(bf16)

    pp = np.arange(128)[:, None]
    jj = np.arange(128)[None, :]
    mk = (jj >= pp).astype(bf16)                      # [128,128] triangle

    eyb = np.eye(64, dtype=np.float32)
    eyf = np.eye(65, dtype=np.float32)

    in_maps = []
    for c in range(8):
        b, role = divmod(c, 2)
        lo_g, hi_g = ROLE_CHUNKS[role]
        others = sorted(set(range(4)) - {lo_g, hi_g})
        pan = [2 * lo_g, 2 * lo_g + 1, 2 * hi_g, 2 * hi_g + 1]
        for o in others:
            pan += [2 * o, 2 * o + 1]

        xT = np.ascontiguousarray(x[b].T)             # [1024, 4096]
        xr = xT.reshape(NCHI, 128, T)                 # [chi, cp, t]
        xt = np.empty((128, NPAN, NCHI, PAN), dtype=bf16)
        for j, pg in enumerate(pan):
            xt[:, j] = xr[:, :, pg * PAN:(pg + 1) * PAN].transpose(1, 0, 2)

        gstart = np.empty(NKB, dtype=np.int64)        # global row of each kb
        for kb in range(NKB):
            gstart[kb] = pan[kb // 4] * PAN + (kb % 4) * 128
        vmask = np.zeros((128, 2 * NKB), dtype=np.float32)
        vmask[:, 0:NKB] = (gstart < (lo_g + 1) * CHUNK).astype(np.float32)[None, :]
        vmask[:, NKB:] = (gstart < (hi_g + 1) * CHUNK).astype(np.float32)[None, :]

        in_maps.append(
            {"xt": xt, "wkv": wkv_t, "vm": vmask, "mk": mk, "eyb": eyb,
             "eyf": eyf}
        )
    return in_maps


_NC = None


def get_nc():
    global _NC
    if _NC is None:
        _NC = build_nc()
    return _NC


def kernel(x, Wk, Wv):
    x = np.asarray(x, dtype=np.float32)
    Wk = np.asarray(Wk, dtype=np.float32)
    Wv = np.asarray(Wv, dtype=np.float32)
    nc = get_nc()
    in_maps = make_inputs(x, Wk, Wv)
    res = run_bass_kernel_spmd(nc, in_maps, list(range(8)))
    out = np.empty((B, T, H), dtype=np.float32)
    for c in range(8):
        b, role = divmod(c, 2)
        lo_g, hi_g = ROLE_CHUNKS[role]
        o = res.results[c]["out"]
        out[b, lo_g * CHUNK:(lo_g + 1) * CHUNK] = o[0:CHUNK]
        out[b, hi_g * CHUNK:(hi_g + 1) * CHUNK] = o[CHUNK:]
    return out
